# revision 57
# baseline (speedup 1.0000x reference)
"""Frame-causal sparse attention block (LN + QKV + masked softmax attention
+ out-proj) on 8 TRN2 NeuronCores.

Sharding: core c handles batch b = c//2 and heads [4*(c%2), 4*(c%2)+4).

v2 redesign around the measured bottlenecks of the v1 pipeline (210us:
PE matmul union 140us at mixed 1.2/2.4GHz clock, Scalar 117us busy of
which exp 91us, Vector 92us):
  - PE work is emitted INTERLEAVED: between every attention k-tile's
    S^T matmul and the (softmax-exp-dependent) A@V matmul, small "filler"
    pieces of QKV / V / out-proj / LayerNorm matmul work are emitted so
    the PE never idles waiting on the Scalar engine's exp. A busy PE
    also holds the fast 2.4GHz p-state (idle gaps drop it to 1.2GHz).
  - Scalar keeps ONLY exp (attention + rstd): qkv copies all moved to
    DVE, and the softmax normalization is reciprocal_approx_fast + mult
    on DVE straight out of PSUM (the denominator is replicated on PSUM
    partitions 0:64 because the custom-DVE recip ucode misreads PSUM at
    partition base 64 on HW).
  - Input DMAs are batched into 8 descriptors (was 26) since each
    dma_start costs ~600ns serialized on the Sync queue; x arrives
    chunk-major so LN(0) starts after ~1.5us. ysend DMAs are issued
    from the GpSimd queue so the in-order Sync queue (which holds the
    RS-gated yrecv->out DMAs) can never stall them.
  - rstd is stored bf16 so the xs multiply hits the DVE fast modes.
Per 512-token chunk j the pipeline computes LN stats (g/b folded into
QKV weights with rank-2 colsum corrections), Q^T/K^T in [channel,token]
layout, V in [token,channel] tiles with appended ones-columns (so A@V
also produces softmax denominators on PSUM partitions 64:128),
block-sparse S^T = K@Q^T at 128-key granularity (frame-causal 64-token
frames; diagonal-tile masked quadrant zeroed on GpSimd), exp on Scalar
(scale 1/8 folded), A@V restricted to the causal query subrange,
DVE-reciprocal normalize, and the out-projection split by OUTPUT DIM
(even cores Y[:,0:256], odd Y[:,256:512]) with a pairwise ReduceScatter
summing the partials directly into each core's dim-half.
Host side only shards/transposes inputs and concatenates dim-halves.
"""

import sys
from collections import deque

import numpy as np

sys.path.insert(0, "/opt/trn_rl_repo")

DIM = 512
HEADS = 8
DH = 64
INNER = 512
T = 2048
B = 4
EPS = 1e-5
NCORES = 8
HPC = 4  # heads per core
CQ = HPC * DH  # 256 channels per core for each of Q, K, V
NT = T // 128  # 16 token tiles
NJ = 4  # 512-token chunks
VSTRIDE = HPC * 128  # 512: per k-tile V block [h(64)|ones(64)] x 4
DHALF = DIM // 2  # out-proj dim half per core

_cache = {}


def _build_nc():
    from contextlib import ExitStack

    import concourse.bacc as bacc
    import concourse.bass as bass
    import concourse.tile as tile
    from concourse import mybir

    f32 = mybir.dt.float32
    bf16 = mybir.dt.bfloat16
    AF = mybir.ActivationFunctionType
    OP = mybir.AluOpType

    # Route every Exp/Ln activation to the one table set that contains both:
    # the default first-match pick splits them across two sets and the
    # rstd ln/exp chain then reloads ACT tables each chunk.
    if not getattr(bacc, "_act_tables_patched", False):
        _orig_get_tables = bacc.get_activation_tables

        def _patched_get_tables(arch):
            tabs = _orig_get_tables(arch)
            both = [
                n
                for n, fns in tabs.items()
                if mybir.ActivationFunctionType.Exp in fns
                and mybir.ActivationFunctionType.Ln in fns
            ]
            if both:
                keep = both[0]
                # Square rides along (used for warmup x^2): it exists in
                # nearly every set, so without stripping it the first-match
                # pick lands elsewhere and Ln/Exp then force a 1.28us
                # ACT_TABLE_LOAD mid-warmup
                tabs = {
                    n: (
                        fns
                        if n == keep
                        else fns
                        - {
                            mybir.ActivationFunctionType.Exp,
                            mybir.ActivationFunctionType.Ln,
                            mybir.ActivationFunctionType.Square,
                        }
                    )
                    for n, fns in tabs.items()
                }
            return tabs

        bacc.get_activation_tables = _patched_get_tables
        bacc._act_tables_patched = True

    nc = bacc.Bacc(
        "TRN2",
        target_bir_lowering=False,
        debug=False,
        num_devices=NCORES,
    )

    # ---- external I/O ----
    x_t = nc.dram_tensor("x_t", [DIM, T], bf16, kind="ExternalInput")
    # [512, 768] = [W'_q(256) | W'_k(256) | W'_v(256)] with LN-g folded in,
    # Q/K/V column blocks each ordered [h0|h1|h2|h3] x 64
    w_qkv_s = nc.dram_tensor("w_qkv_s", [DIM, 3 * CQ], bf16, kind="ExternalInput")
    # LN-fold corrections: csg[c] = sum_d g*W, csb[c] = sum_d b*W
    cs2 = nc.dram_tensor("cs2", [2, 3 * CQ], bf16, kind="ExternalInput")
    # out-proj weight rows of this core's own 4 heads (onorm order), full dim
    w_out_s = nc.dram_tensor("w_out_s", [CQ, DIM], bf16, kind="ExternalInput")
    # half of b_out (both pair cores add b/2; the ReduceScatter sums them)
    b_half = nc.dram_tensor("b_half", [1, DIM], bf16, kind="ExternalInput")
    out_ext = nc.dram_tensor("out", [T, DHALF], bf16, kind="ExternalOutput")

    with tile.TileContext(nc) as tc:
        with ExitStack() as stack:
            constp = stack.enter_context(tc.tile_pool(name="const", bufs=1))
            work = stack.enter_context(tc.tile_pool(name="work", bufs=3))
            epool = stack.enter_context(tc.tile_pool(name="epool", bufs=4))
            small = stack.enter_context(tc.tile_pool(name="small", bufs=2))
            ps_misc = stack.enter_context(
                tc.tile_pool(name="ps_misc", bufs=2, space="PSUM")
            )
            ps_s = stack.enter_context(tc.tile_pool(name="ps_s", bufs=2, space="PSUM"))
            ps_o = stack.enter_context(tc.tile_pool(name="ps_o", bufs=1, space="PSUM"))
            dram = stack.enter_context(tc.tile_pool(name="dram", bufs=1, space="DRAM"))
            xpool = stack.enter_context(tc.tile_pool(name="xpool", bufs=1))
            qkvpool = stack.enter_context(tc.tile_pool(name="qkvpool", bufs=1))
            persist = stack.enter_context(tc.tile_pool(name="persist", bufs=1))

            # ---------- input DMAs: batched descriptors, chunk-0 first ----
            # xw holds x^T d-block-major: [128, d*T + t]
            xw = xpool.tile([128, 4 * T], bf16, name="xw", tag="xw")
            x_src = x_t[:].rearrange("(d p) t -> p d t", p=128)
            w_all = constp.tile([128, 4 * 768], bf16, name="w_all", tag="w_all")
            cs_sb = constp.tile([2, 3 * CQ], bf16)
            wout_all = constp.tile([128, 2 * DIM], bf16, name="wo", tag="wo")
            brep = constp.tile([128, DIM], bf16)
            for jx in range(4):
                sl = slice(jx * 512, (jx + 1) * 512)
                nc.sync.dma_start(
                    xw[:].rearrange("p (d t) -> p d t", t=T)[:, :, sl],
                    x_src[:, :, sl],
                )
                if jx == 0:
                    # Q/K weight columns first: qkv(0) starts ~2us earlier;
                    # the V columns are only needed by v(0) a bit later
                    wv = w_all[:].rearrange("p (d c) -> p d c", c=768)
                    ws = w_qkv_s[:].rearrange("(d p) c -> p d c", p=128)
                    nc.sync.dma_start(wv[:, :, 0:512], ws[:, :, 0:512])
                    nc.sync.dma_start(wv[:, :, 512:768], ws[:, :, 512:768])
                    nc.sync.dma_start(cs_sb[:], cs2[:])
                if jx == 1:
                    # out-proj weights are only needed from proj(0) on —
                    # keep the early Sync/DMA window for x, w_qkv
                    nc.sync.dma_start(
                        wout_all[:].rearrange("p (g c) -> p g c", c=DIM),
                        w_out_s[:].rearrange("(g p) c -> p g c", p=128),
                    )
                    nc.sync.dma_start(brep[:], b_half[:].broadcast_to((128, DIM)))
            ones_bf = constp.tile([128, 128], bf16)
            nc.vector.memset(ones_bf[:], 1.0)
            eps_col = constp.tile([128, 1], f32)
            nc.vector.memset(eps_col[:], EPS)

            # ---------- persistent tensors ----------
            mu_rep = xpool.tile([128, T], f32, name="mu_rep", tag="mu_rep")
            rstd_rep = xpool.tile([128, T], bf16, name="rstd_rep", tag="rstd_rep")
            corr2 = xpool.tile([2, T], bf16, name="corr2", tag="corr2")
            xs = xpool.tile([128, 4 * T], bf16, name="xs", tag="xs")
            # two ping-pong chunk-sized x^2 scratch halves (LN of chunk j
            # uses half j%2; at most two LN chunks are ever in flight)
            xsq = xpool.tile([128, 2 * 2048], bf16, name="xsq", tag="xsq")
            # row1 must be all-ones; single-partition writes at base 1 are
            # illegal, so memset both rows and let the row-0 writes below
            # overwrite it
            nc.vector.memset(corr2[:], 1.0)

            qT = [
                qkvpool.tile([128, T], bf16, name=f"qT{p}", tag=f"qT{p}")
                for p in range(2)
            ]
            kT = [
                qkvpool.tile([128, T], bf16, name=f"kT{p}", tag=f"kT{p}")
                for p in range(2)
            ]
            # V natural [token, channel] per-k-tile blocks: per tile t, head
            # slot h: cols [h*128, h*128+64) all-ones and cols [h*128+64,
            # h*128+128) = V_h, so the A@V matmul replicates the softmax
            # denominator across PSUM partitions 0:64 — the partition base
            # the reciprocal_approx_fast custom-DVE ucode requires (reading
            # PSUM at base 64 returns garbage on HW). Only the ones region
            # is memset.
            v_sb = qkvpool.tile([128, NT * VSTRIDE], bf16, name="v_sb", tag="v_sb")
            nc.vector.memset(
                v_sb[:].rearrange("p (s c) -> p s c", c=128)[:, :, 0:DH], 1.0
            )

            # normalized attention output, pair p's two heads stacked on the
            # partition axis (h0 rows 0:64, h1 rows 64:128)
            onorm = [
                persist.tile([128, T], bf16, name=f"on{p}", tag=f"on{p}")
                for p in range(2)
            ]
            # per-chunk partial-Y send buffer for the pairwise ReduceScatter:
            # rows 0:512 = Y_partial[tokens, 0:256], rows 512:1024 =
            # Y_partial[tokens, 256:512]; the RS sums the pair and leaves
            # this core's dim-half in yrecv (RS cannot write IO tensors)
            ysend = [
                dram.tile([1024, DHALF], bf16, name=f"ys{j}", tag=f"ys{j}")
                for j in range(NJ)
            ]
            yrecv = [
                dram.tile([512, DHALF], bf16, name=f"yr{j}", tag=f"yr{j}")
                for j in range(NJ)
            ]

            def xwd(d, lo, hi):
                return xw[:, d * T + lo : d * T + hi]

            def xsd(d, lo, hi):
                return xs[:, d * T + lo : d * T + hi]

            # ---------- LayerNorm pieces ----------
            def ln_pieces(j):
                cl = slice(j * 512, (j + 1) * 512)
                xq = xsq[:, (j % 2) * 2048 : (j % 2) * 2048 + 2048]
                st = {}

                def p1():
                    s1t = ps_misc.tile([128, 512], f32, name="s1t", tag="m")
                    s1 = s1t[0:1, :]
                    for d in range(4):
                        nc.tensor.matmul(
                            s1,
                            ones_bf[:, 0:1],
                            xwd(d, j * 512, (j + 1) * 512),
                            start=(d == 0),
                            stop=(d == 3),
                            skip_group_check=True,
                        )
                    sv1 = small.tile([1, 512], bf16, name="sv1", tag="sv1")
                    st["sv1"] = sv1
                    nc.vector.tensor_copy(sv1[:], s1)
                    for d in range(4):
                        if j < 2:
                            # warmup chunks: Scalar is idle (no exp yet) and
                            # the DVE chain is the pipeline-fill critical
                            # path; Square shares the Exp/Ln ACT table set
                            nc.scalar.activation(
                                xq[:, d * 512 : (d + 1) * 512],
                                xwd(d, j * 512, (j + 1) * 512),
                                AF.Square,
                                bias=0.0,
                                scale=1.0,
                            )
                        else:
                            nc.vector.tensor_tensor(
                                xq[:, d * 512 : (d + 1) * 512],
                                xwd(d, j * 512, (j + 1) * 512),
                                xwd(d, j * 512, (j + 1) * 512),
                                OP.mult,
                            )

                def p2():
                    s2t = ps_misc.tile([128, 512], f32, name="s2t", tag="m")
                    s2 = s2t[0:1, :]
                    for d in range(4):
                        nc.tensor.matmul(
                            s2,
                            ones_bf[:, 0:1],
                            xq[:, d * 512 : (d + 1) * 512],
                            start=(d == 0),
                            stop=(d == 3),
                            skip_group_check=True,
                        )
                    sv2 = small.tile([1, 512], bf16, name="sv2", tag="sv2")
                    st["sv2"] = sv2
                    nc.vector.tensor_copy(sv2[:], s2)

                def p3():
                    rep1 = ps_misc.tile([128, 512], f32, name="rep1", tag="m")
                    rep2 = ps_misc.tile([128, 512], f32, name="rep2", tag="m")
                    nc.tensor.matmul(
                        rep1[:], ones_bf[0:1, :], st["sv1"][:], skip_group_check=True
                    )
                    nc.tensor.matmul(
                        rep2[:], ones_bf[0:1, :], st["sv2"][:], skip_group_check=True
                    )
                    nc.vector.tensor_scalar(
                        mu_rep[:, cl], rep1[:], 1.0 / DIM, None, OP.mult
                    )
                    musq = work.tile([128, 512], f32, name="musq", tag="musq")
                    nc.vector.tensor_tensor(
                        musq[:], mu_rep[:, cl], mu_rep[:, cl], OP.mult
                    )
                    var = work.tile([128, 512], f32, name="var", tag="var")
                    nc.vector.scalar_tensor_tensor(
                        var[:], rep2[:], 1.0 / DIM, musq[:], OP.mult, OP.subtract
                    )
                    lnv = work.tile([128, 512], f32, name="lnv", tag="lnv")
                    nc.scalar.activation(
                        lnv[:], var[:], AF.Ln, bias=eps_col[:], scale=1.0
                    )
                    nc.scalar.activation(
                        rstd_rep[:, cl], lnv[:], AF.Exp, bias=0.0, scale=-0.5
                    )
                    nc.vector.scalar_tensor_tensor(
                        corr2[0:1, cl],
                        mu_rep[0:1, cl],
                        -1.0,
                        rstd_rep[0:1, cl],
                        OP.mult,
                        OP.mult,
                    )
                    for d in range(4):
                        nc.vector.tensor_tensor(
                            xsd(d, j * 512, (j + 1) * 512),
                            xwd(d, j * 512, (j + 1) * 512),
                            rstd_rep[:, cl],
                            OP.mult,
                        )

                return [p1, p2, p3]

            # ---------- QKV pieces (Q^T / K^T columns) ----------
            def qkv_pieces(j):
                cl = slice(j * 512, (j + 1) * 512)
                out = []
                # pair-0 tensors (ct 0: qT0, ct 2: kT0) first so the next
                # chunk's pair-0 attention can start before pair-1 copies land
                for ct in (0, 2, 1, 3):  # 0,1 -> Q pairs; 2,3 -> K pairs

                    def f(ct=ct):
                        dst = qT[ct] if ct < 2 else kT[ct - 2]
                        acc = ps_misc.tile([128, 512], f32, name="qkv_ps", tag="m")
                        for d in range(4):
                            nc.tensor.matmul(
                                acc[:],
                                w_all[:, d * 768 + ct * 128 : d * 768 + (ct + 1) * 128],
                                xsd(d, j * 512, (j + 1) * 512),
                                start=(d == 0),
                                stop=False,
                                skip_group_check=True,
                            )
                        nc.tensor.matmul(
                            acc[:],
                            cs_sb[:, ct * 128 : (ct + 1) * 128],
                            corr2[:, cl],
                            start=False,
                            stop=True,
                            skip_group_check=True,
                        )
                        nc.vector.tensor_copy(dst[:, cl], acc[:])

                    out.append(f)
                return out

            # ---------- V pieces ----------
            def v_pieces(j):
                out = []
                for tt in range(4 * j, 4 * j + 4):

                    def f(tt=tt):
                        tl = slice(tt * 128, (tt + 1) * 128)
                        vact = ps_misc.tile([128, 512], f32, name="v_ps", tag="m")
                        vac = vact[:, 0:CQ]
                        for d in range(4):
                            nc.tensor.matmul(
                                vac,
                                xsd(d, tt * 128, (tt + 1) * 128),
                                w_all[:, d * 768 + 512 : d * 768 + 768],
                                start=(d == 0),
                                stop=False,
                                skip_group_check=True,
                            )
                        nc.tensor.matmul(
                            vac,
                            corr2[:, tl],
                            cs_sb[:, 2 * CQ : 3 * CQ],
                            start=False,
                            stop=True,
                            skip_group_check=True,
                        )
                        dst = v_sb[:, tt * VSTRIDE : (tt + 1) * VSTRIDE].rearrange(
                            "p (h c) -> p h c", c=128
                        )[:, :, DH:128]
                        nc.vector.tensor_copy(
                            dst, vac.rearrange("p (h c) -> p h c", c=DH)
                        )

                    out.append(f)
                return out

            # ---------- out-projection pieces ----------
            def proj_pieces(j):
                out = []
                for tt in range(4 * j, 4 * j + 4):

                    def f(tt=tt):
                        tl = slice(tt * 128, (tt + 1) * 128)
                        ops = ps_misc.tile([128, 512], f32, name="out_ps", tag="m")
                        for p in range(2):
                            nc.tensor.matmul(
                                ops[:],
                                onorm[p][:, tl],
                                wout_all[:, p * DIM : (p + 1) * DIM],
                                start=(p == 0),
                                stop=(p == 1),
                                skip_group_check=True,
                            )
                        ostage = work.tile([128, DIM], bf16, name="ostage", tag="ost")
                        nc.vector.tensor_tensor(ostage[:], ops[:], brep[:], OP.add)
                        rr = slice((tt - 4 * j) * 128, (tt - 4 * j + 1) * 128)
                        # issued from GpSimd, not Sync: the Sync queue holds
                        # the yrecv->out DMAs which wait on a 10-30us RS, and
                        # in-order queueing would stall these sends (and via
                        # the ostage WAR, the whole pipeline) behind them
                        nc.gpsimd.dma_start(
                            ysend[j][:].rearrange("(b r) c -> r b c", b=2)[rr],
                            ostage[:].rearrange("p (b c) -> p b c", b=2),
                        )

                    out.append(f)

                # the RS trigger is returned as a SEPARATE piece: it must be
                # emitted only after its ysend DMAs are long done, or its
                # dependency wait blocks the GpSimd queue and stalls the
                # diag-quadrant memsets that attention A@V waits on.
                # The yrecv->out_ext DMA is NOT issued here: it waits on the
                # RS (10-30us) and the Sync DMA queue is in-order, so issuing
                # it mid-run blocks every later ysend DMA (and through the
                # ostage WAR, the whole pipeline). All out DMAs go at the end.
                def rs(j=j):
                    from concourse import mybir as mb

                    nc.gpsimd.collective_compute(
                        "ReduceScatter",
                        mb.AluOpType.add,
                        replica_groups=[[2 * b, 2 * b + 1] for b in range(B)],
                        ins=[ysend[j][:].opt()],
                        outs=[yrecv[j][:].opt()],
                    )
                    nc.sync.dma_start(out_ext[j * 512 : (j + 1) * 512, :], yrecv[j][:])

                return out, rs

            # ---------- attention ----------
            def attn_chunk(p, j, fill, slots, half_proj=None):
                # half_proj (last chunk, pair 1 only): [4 proj tile pieces].
                # o_ps column block [0,256) is final once the AV of diag tile
                # 4j+1 has run, so its normalize + proj tiles overlap the
                # remaining diagonal A@Vs instead of serializing the tail.
                # separate per-head PSUM tiles: a merged tile looks like a
                # WAW conflict between the heads' A@V writes to the tile
                # tracker and the extra sem ordering costs ~24us of PE time
                o_ps = [
                    ps_o.tile([128, 512], f32, name=f"o_ps{h}", tag=f"o_ps{h}")
                    for h in range(2)
                ]
                nkt = 4 * (j + 1)  # k tiles participating

                def av(i, e_sb, off, n, diag):
                    # A@V restricted to the causal query subrange; columns
                    # [0, off) of o_ps were initialized by the full-width
                    # i=0 matmul, so subranges accumulate with start=False
                    for h in range(2):
                        vblk = v_sb[
                            :,
                            i * VSTRIDE
                            + (2 * p + h) * 128 : i * VSTRIDE
                            + (2 * p + h + 1) * 128,
                        ]
                        nc.tensor.matmul(
                            o_ps[h][:, off : off + n],
                            vblk[:],
                            e_sb[:, h * 512 + off : h * 512 + off + n],
                            start=(i == 0),
                            stop=(i == nkt - 1),
                            skip_group_check=True,
                        )

                cl = slice(j * 512, (j + 1) * 512)

                def norm_half(lo, hi):
                    for h in range(2):
                        rec = small.tile(
                            [64, hi - lo], f32, name="rec", tag=f"rec{hi - lo}"
                        )
                        nc.vector.reciprocal_approx_fast(
                            rec[:], o_ps[h][0:64, lo:hi]
                        )
                        nc.vector.tensor_tensor(
                            onorm[p][h * 64 : (h + 1) * 64, j * 512 + lo : j * 512 + hi],
                            o_ps[h][64:128, lo:hi],
                            rec[:],
                            OP.mult,
                        )

                pend = None  # software pipeline: A@V lags one k-tile
                for i in range(nkt):
                    q0 = max(512 * j, 128 * i)
                    n = 512 * (j + 1) - q0
                    off = q0 - 512 * j
                    diag = q0 == 128 * i
                    s_ps = ps_s.tile([128, 1024], f32, name="s_ps", tag="s_ps")
                    for h in range(2):
                        hr = slice(h * 64, (h + 1) * 64)
                        nc.tensor.matmul(
                            s_ps[:, h * 512 + off : h * 512 + off + n],
                            kT[p][hr, i * 128 : (i + 1) * 128],
                            qT[p][hr, q0 : q0 + n],
                            skip_group_check=True,
                        )

                    e_sb = epool.tile([128, 1024], bf16, name="e_sb", tag="e_sb")
                    sr = s_ps[:].rearrange("p (x n) -> p x n", x=2)[
                        :, :, off : off + n
                    ]
                    er = e_sb[:].rearrange("p (x n) -> p x n", x=2)[
                        :, :, off : off + n
                    ]
                    nc.scalar.activation(er, sr, AF.Exp, bias=0.0, scale=0.125)
                    if diag:
                        # frame-causal quadrant: key rows 64:128 (frame
                        # 2i+1) must not contribute to queries 128i..128i+64
                        eq = e_sb[64:128, :].rearrange(
                            "p (x n) -> p x n", x=2
                        )[:, :, off : off + 64]
                        nc.gpsimd.memset(eq, 0.0)
                    # filler PE work rides in the exp-shadow of this tile
                    if fill:
                        k = -(-len(fill) // max(slots[0], 1))
                        for _ in range(min(k, len(fill))):
                            fill.popleft()()
                    slots[0] -= 1
                    if pend is not None:
                        av(*pend)
                        if half_proj is not None and pend[0] == 4 * j + 1:
                            # cols [0,256) of o_ps are final: normalize and
                            # project them under the remaining diag A@Vs
                            norm_half(0, 256)
                            half_proj[0]()
                            half_proj[1]()
                    pend = (i, e_sb, off, n, diag)
                av(*pend)
                if half_proj is not None:
                    norm_half(256, 512)
                    half_proj[2]()
                    half_proj[3]()
                    return
                # normalize: rows 0:64 (per-head o) times the reciprocal of
                # the denominator replicated on PSUM partitions 64:128.
                # Two DVE ops per head (recip_approx is ~5x faster than the
                # reciprocal ucode; 18 bits is plenty for a bf16 result);
                # each op reads at most one PSUM operand (NCC_IBVF027).
                norm_half(0, 512)

            # ---------- schedule ----------
            # chunk-0 LN chain first and straight into qkv(0): emitting
            # ln1's stats matmul earlier blocks the in-order PE queue on the
            # x-chunk-1 DMA (~+16us) while ready chunk-0 work sits behind it
            ln0 = ln_pieces(0)
            ln1 = ln_pieces(1)
            ln0[0]()
            ln0[1]()
            ln0[2]()
            for f in qkv_pieces(0):
                f()
            ln1[0]()
            ln1[1]()
            ln1[2]()
            for f in v_pieces(0):
                f()
            # out-proj work is deliberately back-loaded (proj(0) in chunk 2,
            # proj(1)+proj(2) in chunk 3): the late chunks have the most
            # attention tiles and the least remaining QKV/LN filler, and the
            # per-chunk ReduceScatters still land ~10us apart on the CC queue
            for j in range(NJ):
                fill = deque()
                rss = []
                if j in (0, 1):
                    # next chunk's Q^T/K^T first: the chunk boundary stalls
                    # on their PSUM->SBUF copies otherwise. (For j=2 the
                    # qkv(3) pieces depend on ln(3) emitted in this same
                    # chunk, so they must come after it.)
                    fill.extend(qkv_pieces(j + 1))
                if j == 0:
                    fill.extend(ln_pieces(2))
                if j == 2:
                    fill.extend(ln_pieces(3))
                    pp, rs = proj_pieces(0)
                    fill.extend(pp)
                    rss.append(rs)
                    fill.extend(qkv_pieces(3))
                if j == 3:
                    pp, rs = proj_pieces(1)
                    fill.extend(pp)
                    rss.append(rs)
                    pp, rs = proj_pieces(2)
                    fill.extend(pp)
                    rss.append(rs)
                if j + 1 < NJ:
                    fill.extend(v_pieces(j + 1))
                fill.extend(rss)
                slots = [8 * (j + 1)]
                attn_chunk(0, j, fill, slots)
                if j == NJ - 1:
                    pp3, rs3 = proj_pieces(NJ - 1)
                    attn_chunk(1, j, fill, slots, half_proj=pp3)
                else:
                    attn_chunk(1, j, fill, slots)
                while fill:
                    fill.popleft()()
            rs3()

    nc.compile()
    return nc


def _prep_in_maps(x, ln_g, ln_b, w_qkv, w_out, b_out):
    import ml_dtypes

    bf = ml_dtypes.bfloat16
    wp = ln_g[:, None] * w_qkv  # [512, 1536]
    csg = wp.sum(axis=0)  # [1536]
    csb = (ln_b[:, None] * w_qkv).sum(axis=0)
    in_maps = []
    for c in range(NCORES):
        b = c // 2
        heads = range(4 * (c % 2), 4 * (c % 2) + 4)
        qcols = np.concatenate([np.arange(h * DH, (h + 1) * DH) for h in heads])
        cols = np.concatenate([qcols, INNER + qcols, 2 * INNER + qcols])
        # w_out rows of this core's own 4 heads, onorm order (pair-major)
        wrows = np.concatenate([np.arange(h * DH, (h + 1) * DH) for h in heads])
        in_maps.append(
            {
                "x_t": np.ascontiguousarray(x[b].T).astype(bf),
                "w_qkv_s": np.ascontiguousarray(wp[:, cols]).astype(bf),
                "cs2": np.ascontiguousarray(np.stack([csg[cols], csb[cols]])).astype(
                    bf
                ),
                "w_out_s": np.ascontiguousarray(w_out[wrows, :]).astype(bf),
                "b_half": (b_out / 2).reshape(1, DIM).astype(bf),
            }
        )
    return in_maps


def _run(inputs, trace=False):
    from concourse.bass_utils import run_bass_kernel_spmd

    if "nc" not in _cache:
        _cache["nc"] = _build_nc()
    nc = _cache["nc"]
    in_maps = _prep_in_maps(
        np.asarray(inputs["x"], dtype=np.float32),
        np.asarray(inputs["ln_g"], dtype=np.float32),
        np.asarray(inputs["ln_b"], dtype=np.float32),
        np.asarray(inputs["w_qkv"], dtype=np.float32),
        np.asarray(inputs["w_out"], dtype=np.float32),
        np.asarray(inputs["b_out"], dtype=np.float32),
    )
    res = run_bass_kernel_spmd(nc, in_maps, core_ids=list(range(NCORES)), trace=trace)
    out = np.empty((B, T, DIM), dtype=np.float32)
    for b in range(B):
        out[b, :, 0:DHALF] = res.results[2 * b]["out"].astype(np.float32)
        out[b, :, DHALF:DIM] = res.results[2 * b + 1]["out"].astype(np.float32)
    return out, res


def kernel(**inputs):
    return _run(inputs, trace=False)[0]


def kernel_traced(**inputs):
    out, res = _run(inputs, trace=True)
    return out, res
